# revision 1
# baseline (speedup 1.0000x reference)
"""GQA attention (qk-rmsnorm + partial RoPE) on 8 trn2 NeuronCores.

Sharding: sequence-parallel. B*S = 4096 rows split 8 ways (512 rows/core,
cores 0-3 = batch 0, cores 4-7 = batch 1). Each core projects q/k/v for its
rows (full head width, so the full-dim rmsnorm stays local), norms + ropes
its k rows, and AllGathers post-norm K / V across its 4-core batch group.
Attention + output projection are then fully local; the row-sharded outputs
are concatenated on the host.

Compute dtype bf16 (f32 psum accumulate, f32 softmax stats).
"""

import numpy as np
import ml_dtypes
from contextlib import ExitStack

import concourse.bass as bass
import concourse.tile as tile
from concourse import mybir, bacc
from concourse.bass_utils import run_bass_kernel_spmd
from concourse.masks import make_identity

B, S, H = 2, 2048, 4096
NQ, NK, D, RD = 32, 8, 128, 64
HALF = RD // 2
EPS = 1e-6
NCORES = 8
GRP = 4                      # cores per batch group
R = B * S // NCORES          # 512 rows per core
SCALE = D ** -0.5
BF16 = mybir.dt.bfloat16
F32 = mybir.dt.float32
NHC = H // 128               # 32 contraction chunks
KC = (GRP * R) // 128        # 16 k-row chunks per batch

_cache = {}


def _build():
    nc = bacc.Bacc("TRN2", target_bir_lowering=False, debug=False,
                   num_devices=NCORES)
    di = lambda n, s, d: nc.dram_tensor(n, s, d, kind="ExternalInput").ap()
    xT = di("xT", [H, R], BF16)
    wq = di("wq", [H, NQ * D], BF16)
    wk = di("wk", [H, NK * D], BF16)
    wv = di("wv", [H, NK * D], BF16)
    wo = di("wo", [NQ * D, H], BF16)
    cosT = di("cosT", [RD, R], BF16)
    sinTs = di("sinTs", [RD, R], BF16)        # rows 0:32 = -sinT, 32:64 = +sinT
    invgq2 = di("invgq2", [NQ * D, 1], BF16)  # 1/gq^2 (ssq weights)
    invgk2 = di("invgk2", [NK * D, 1], BF16)
    out = nc.dram_tensor("out", [R, H], F32, kind="ExternalOutput").ap()

    with tile.TileContext(nc) as tc:
        _emit(nc, tc, xT, wq, wk, wv, wo, cosT, sinTs, invgq2, invgk2, out)
    nc.finalize()
    return nc


def _emit(nc, tc, xT, wq, wk, wv, wo, cosT, sinTs, invgq2, invgk2, out):
    ctx = ExitStack()
    singles = ctx.enter_context(tc.tile_pool(name="singles", bufs=1))
    dram = ctx.enter_context(tc.tile_pool(name="dram", bufs=4, space="DRAM"))
    small = ctx.enter_context(tc.tile_pool(name="small", bufs=4))

    # ---- resident small inputs ----
    cos_sb = singles.tile([RD, R], BF16)
    nc.sync.dma_start(out=cos_sb, in_=cosT[:, :])
    sin_sb = singles.tile([RD, R], BF16)
    nc.sync.dma_start(out=sin_sb, in_=sinTs[:, :])
    igq_sb = singles.tile([128, NQ], BF16)
    nc.sync.dma_start(out=igq_sb, in_=invgq2.rearrange("(j p) o -> p (j o)", p=128))
    igk_sb = singles.tile([128, NK], BF16)
    nc.sync.dma_start(out=igk_sb, in_=invgk2.rearrange("(j p) o -> p (j o)", p=128))
    ident = singles.tile([128, 128], BF16)
    make_identity(nc, ident)
    eps1 = singles.tile([1, 1], F32)
    nc.vector.memset(eps1, EPS)
    ones_sb = singles.tile([128, 1], BF16)
    nc.vector.memset(ones_sb, 1.0)

    attkv = ctx.enter_context(tc.tile_pool(name="attkv", bufs=1))
    kT_full = attkv.tile([128, NK, GRP, R], BF16)     # [d, h, j, r]
    qpool_cm = tc.tile_pool(name="qpool", bufs=1)
    qpool = qpool_cm.__enter__()
    qts = [qpool.tile([128, R], BF16, name=f"qh{c}", tag=f"qh{c}")
           for c in range(NQ)]

    # ---- AllGather bounce buffers ----
    agk_in = dram.tile([128, NK, R], BF16, tag="agki")
    agk_out = dram.tile([GRP, 128, NK, R], BF16, tag="agko")
    agv_in = dram.tile([128, R // 128, NK, D], BF16, tag="agvi")
    agv_out = dram.tile([GRP, 128, R // 128, NK, D], BF16, tag="agvo")
    rgroups = [[0, 1, 2, 3], [4, 5, 6, 7]]

    def make_scale_bc(ssq_ps, inv_n, pool):
        """rsqrt(ssq/n + eps), broadcast to [128, R] f32 via DRAM bounce."""
        r1 = small.tile([1, R], F32, tag="r1")
        nc.scalar.activation(out=r1, in_=ssq_ps,
                             func=mybir.ActivationFunctionType.Sqrt,
                             bias=eps1, scale=inv_n)
        nc.vector.reciprocal(out=r1, in_=r1)
        sc_d = dram.tile([1, R], F32, tag="scd")
        nc.sync.dma_start(out=sc_d, in_=r1)
        bc = pool.tile([128, R], F32, tag="scale_bc")
        nc.sync.dma_start(out=bc, in_=sc_d.to_broadcast([128, R]))
        return bc

    # =============== projections + norm + rope + AllGather ================
    with tc.tile_pool(name="kvtmp", bufs=1) as kvtmp, \
         tc.tile_pool(name="wpool", bufs=3) as wpool, \
         tc.tile_pool(name="sqpool", bufs=3) as sqpool, \
         tc.tile_pool(name="shp", bufs=3) as shp, \
         tc.tile_pool(name="pps", bufs=3, space="PSUM") as pps, \
         tc.tile_pool(name="projacc", bufs=1, space="PSUM") as projacc, \
         tc.tile_pool(name="tps", bufs=2, space="PSUM") as tps:

        kT_all = kvtmp.tile([128, NK, R], BF16)   # roped/normed in place
        V_own = kvtmp.tile([128, R // 128, NK, D], BF16)   # [p, rc, h, d]
        xT_sb = kvtmp.tile([128, NHC, R], BF16)
        nc.sync.dma_start(out=xT_sb, in_=xT.rearrange("(j p) r -> p j r", p=128))

        pending = []

        def flush_ssq():
            while pending:
                ssq_tile, lhs, sq, first, last = pending.pop(0)
                nc.tensor.matmul(ssq_tile, lhs, sq,
                                 start=first, stop=last, skip_group_check=True)

        def proj_chunk(w_ap, c, ssq_lhs, ssq_tile, first, last, dst_ap):
            wsb = wpool.tile([128, NHC, 128], BF16, tag="wslab")
            nc.sync.dma_start(
                out=wsb,
                in_=w_ap[:, c * 128:(c + 1) * 128].rearrange("(j p) f -> p j f", p=128))
            ps = pps.tile([128, R], F32, tag="proj")
            for j in range(NHC):
                nc.tensor.matmul(ps, wsb[:, j, :], xT_sb[:, j, :],
                                 start=(j == 0), stop=(j == NHC - 1))
            nc.scalar.copy(out=dst_ap, in_=ps)
            flush_ssq()
            if ssq_tile is not None:
                sq = sqpool.tile([128, R], BF16, tag="sq")
                nc.vector.tensor_mul(sq, dst_ap, dst_ap)
                pending.append((ssq_tile, ssq_lhs[:, c:c + 1], sq, first, last))

        def rope_norm_inplace(buf_ap, bc):
            """buf = rope(buf) * bc  (rope on partitions 0:RD), in place."""
            sh = shp.tile([RD, R], BF16, tag="shift")
            nc.scalar.dma_start(out=sh[0:HALF, :], in_=buf_ap[HALF:RD, :])
            nc.scalar.dma_start(out=sh[HALF:RD, :], in_=buf_ap[0:HALF, :])
            nc.vector.tensor_mul(sh, sh, sin_sb)
            t2 = shp.tile([RD, R], BF16, tag="tcos")
            nc.vector.tensor_mul(t2, buf_ap[0:RD, :], cos_sb)
            nc.vector.tensor_add(buf_ap[0:RD, :], t2, sh)
            nc.vector.tensor_mul(buf_ap, buf_ap, bc)

        ssqk = projacc.tile([1, R], F32, tag="ssqk")
        for c in range(NK):
            proj_chunk(wk, c, igk_sb, ssqk, c == 0, c == NK - 1, kT_all[:, c, :])
        for c in range(NK):
            vt = sqpool.tile([128, R], BF16, tag="vtchunk")
            proj_chunk(wv, c, None, None, False, False, vt[:, :])
            for rc in range(R // 128):
                tp = tps.tile([128, 128], BF16, tag="vtp")
                nc.tensor.transpose(tp, vt[:, rc * 128:(rc + 1) * 128], ident)
                nc.scalar.copy(out=V_own[:, rc, c, :], in_=tp)
        flush_ssq()
        rk_bc = make_scale_bc(ssqk, 1.0 / (NK * D), kvtmp)
        for c in range(NK):
            rope_norm_inplace(kT_all[:, c, :], rk_bc)
        # ship post-norm k/v to the batch group (overlaps q projection)
        nc.gpsimd.dma_start(out=agk_in[:], in_=kT_all)
        nc.gpsimd.dma_start(out=agv_in[:], in_=V_own)
        nc.gpsimd.collective_compute(
            "AllGather", mybir.AluOpType.bypass, replica_groups=rgroups,
            ins=[agk_in.opt()], outs=[agk_out.opt()])
        nc.gpsimd.collective_compute(
            "AllGather", mybir.AluOpType.bypass, replica_groups=rgroups,
            ins=[agv_in.opt()], outs=[agv_out.opt()])
        for j in range(GRP):
            nc.sync.dma_start(out=kT_full[:, :, j, :], in_=agk_out[j])

        ssqq = projacc.tile([1, R], F32, tag="ssqq")
        for c in range(NQ):
            proj_chunk(wq, c, igq_sb, ssqq, c == 0, c == NQ - 1, qts[c][:, :])
        flush_ssq()
        rq_bc = make_scale_bc(ssqq, 1.0 / (NQ * D), kvtmp)
        for c in range(NQ):
            rope_norm_inplace(qts[c][:, :], rq_bc)

    # ========================== attention =================================
    aots = [singles.tile([128, R], BF16, name=f"ao{c}", tag=f"ao{c}")
            for c in range(NQ)]
    with tc.tile_pool(name="vfp", bufs=1) as vfp, \
         tc.tile_pool(name="expp", bufs=3) as expp, \
         tc.tile_pool(name="recp", bufs=3) as recp, \
         tc.tile_pool(name="attps", bufs=4, space="PSUM") as attps, \
         tc.tile_pool(name="accps", bufs=2, space="PSUM") as accps:
        V_full = vfp.tile([128, KC, NK, D], BF16)     # [p, kc, h, d]
        for j in range(GRP):
            nc.sync.dma_start(
                out=V_full[:, j * (R // 128):(j + 1) * (R // 128), :, :],
                in_=agv_out[j])
        for qh in range(NQ):
            kvh = qh // GRP
            ex = expp.tile([128, KC, R], BF16, tag="expT")
            sum_ps = accps.tile([1, R], F32, tag="sumexp")
            out_ps = accps.tile([128, R], F32, tag="pv")
            for kc in range(KC):
                sps = attps.tile([128, R], F32, tag="scoresT")
                nc.tensor.matmul(
                    sps,
                    kT_full[:, kvh, kc // 4, (kc % 4) * 128:(kc % 4) * 128 + 128],
                    qts[qh][:, :], start=True, stop=True)
                nc.scalar.activation(out=ex[:, kc, :], in_=sps,
                                     func=mybir.ActivationFunctionType.Exp,
                                     scale=SCALE)
            for kc in range(KC):
                nc.tensor.matmul(sum_ps, ones_sb, ex[:, kc, :],
                                 start=(kc == 0), stop=(kc == KC - 1),
                                 skip_group_check=True)
            for kc in range(KC):
                nc.tensor.matmul(out_ps, V_full[:, kc, kvh, :], ex[:, kc, :],
                                 start=(kc == 0), stop=(kc == KC - 1),
                                 skip_group_check=True)
            rec = small.tile([1, R], F32, tag="rec")
            nc.vector.reciprocal(out=rec, in_=sum_ps)
            rec_d = dram.tile([1, R], F32, tag="recd")
            nc.scalar.dma_start(out=rec_d, in_=rec)
            rec_bc = recp.tile([128, R], F32, tag="rec_bc")
            nc.scalar.dma_start(out=rec_bc, in_=rec_d.to_broadcast([128, R]))
            nc.vector.tensor_mul(aots[qh][:, :], out_ps, rec_bc)

    qpool_cm.__exit__(None, None, None)

    # ======================= output projection ============================
    # attn_oT stationary across all H-blocks of a pair: 128 LDWs total.
    with tc.tile_pool(name="wop", bufs=2) as wop, \
         tc.tile_pool(name="odr", bufs=4) as odr, \
         tc.tile_pool(name="ops", bufs=1, space="PSUM") as ops:
        NHP = H // 512
        for hp in range(NHP):
            wosb = wop.tile([128, NQ, 512], BF16, tag="wo")
            nc.sync.dma_start(
                out=wosb,
                in_=wo[:, hp * 512:(hp + 1) * 512].rearrange(
                    "(j p) f -> p j f", p=128))
            pos = [ops.tile([128, 512], F32, tag=f"ops{i}", name=f"po{hp}_{i}")
                   for i in range(4)]
            for h in range(NQ):
                for rc in range(R // 128):
                    nc.tensor.matmul(
                        pos[rc],
                        aots[h][:, rc * 128:(rc + 1) * 128],
                        wosb[:, h, :],
                        start=(h == 0), stop=(h == NQ - 1),
                        skip_group_check=True)
            for rc in range(R // 128):
                osb = odr.tile([128, 512], F32, tag="odr")
                nc.scalar.copy(out=osb, in_=pos[rc])
                nc.sync.dma_start(
                    out=out[rc * 128:(rc + 1) * 128, hp * 512:(hp + 1) * 512],
                    in_=osb)
    ctx.close()


def _get_nc():
    if "nc" not in _cache:
        _cache["nc"] = _build()
    return _cache["nc"]


def kernel(x, cos, sin, wq, wk, wv, wo, gq, gk):
    bf = ml_dtypes.bfloat16
    x = np.asarray(x, np.float32)
    cos = np.asarray(cos, np.float32)
    sin = np.asarray(sin, np.float32)
    gq = np.asarray(gq, np.float32)
    gk = np.asarray(gk, np.float32)
    wqp = (np.asarray(wq, np.float32) * gq[None, :]).astype(bf)
    wkp = (np.asarray(wk, np.float32) * gk[None, :]).astype(bf)
    wv_b = np.asarray(wv, np.float32).astype(bf)
    wo_b = np.asarray(wo, np.float32).astype(bf)
    igq = np.where(gq == 0, 0, 1.0 / np.maximum(gq * gq, 1e-30)).astype(bf)[:, None]
    igk = np.where(gk == 0, 0, 1.0 / np.maximum(gk * gk, 1e-30)).astype(bf)[:, None]

    x2 = x.reshape(B * S, H)
    in_maps = []
    for c in range(NCORES):
        p0 = (c % GRP) * R
        sinT = sin[p0:p0 + R].T.astype(np.float32)        # [RD, R]
        sinTs = np.concatenate([-sinT[:HALF], sinT[HALF:]], 0)
        in_maps.append({
            "xT": np.ascontiguousarray(x2[c * R:(c + 1) * R].T).astype(bf),
            "wq": wqp, "wk": wkp, "wv": wv_b, "wo": wo_b,
            "cosT": np.ascontiguousarray(cos[p0:p0 + R].T).astype(bf),
            "sinTs": np.ascontiguousarray(sinTs).astype(bf),
            "invgq2": igq, "invgk2": igk,
        })
    nc = _get_nc()
    import os
    kw = {}
    if os.environ.get("KERNEL_TRACE"):
        kw = dict(trace=True, tmpdir=os.environ.get("KERNEL_TRACE_DIR") or None)
    res = run_bass_kernel_spmd(nc, in_maps, core_ids=list(range(NCORES)), **kw)
    kernel.last_exec_time_ns = res.exec_time_ns
    outp = np.concatenate([res.results[c]["out"] for c in range(NCORES)], 0)
    return outp.reshape(B, S, H).astype(np.float32)



# revision 15
# speedup vs baseline: 1.0928x; 1.0928x over previous
"""GQA attention (qk-rmsnorm + partial RoPE) on 8 trn2 NeuronCores.

Sharding: sequence-parallel. B*S = 4096 rows split 8 ways (512 rows/core,
cores 0-3 = batch 0, cores 4-7 = batch 1). Each core projects q/k/v for its
rows (full head width, so the full-dim rmsnorm stays local), norms + ropes
its k rows, and AllGathers post-norm K / V across its 4-core batch group.
Attention + output projection are then fully local; the row-sharded outputs
are concatenated on the host.

v2 vs baseline: sumexp moved off the tensor engine (gpsimd+vector tree +
one rank-1 matmul per head), exp batched 2 chunks/instr, rsqrt via Ln/Exp
on scalar, partition_broadcast instead of DRAM-bounce broadcasts, per-head
software-pipelined attention tail, split xT DMA for a fast start.
"""

import numpy as np
import ml_dtypes
from contextlib import ExitStack

import concourse.bass as bass
import concourse.tile as tile
from concourse import mybir, bacc
from concourse.bass_utils import run_bass_kernel_spmd
from concourse.masks import make_identity
from concourse import bass_isa

B, S, H = 2, 2048, 4096
NQ, NK, D, RD = 32, 8, 128, 64
HALF = RD // 2
EPS = 1e-6
NCORES = 8
GRP = 4                      # cores per batch group
R = B * S // NCORES          # 512 rows per core
SCALE = D ** -0.5
BF16 = mybir.dt.bfloat16
F32 = mybir.dt.float32
NHC = H // 128               # 32 contraction chunks
KC = (GRP * R) // 128        # 16 k-row chunks per batch

_cache = {}


def _build():
    nc = bacc.Bacc("TRN2", target_bir_lowering=False, debug=False,
                   num_devices=NCORES)
    di = lambda n, s, d: nc.dram_tensor(n, s, d, kind="ExternalInput").ap()
    xT = di("xT", [H, R], BF16)
    wq = di("wq", [H, NQ * D], BF16)
    wk = di("wk", [H, NK * D], BF16)
    wv = di("wv", [H, NK * D], BF16)
    wo = di("wo", [NQ * D, H], BF16)
    cosT = di("cosT", [RD, R], BF16)
    sinTs = di("sinTs", [RD, R], BF16)        # rows 0:32 = -sinT, 32:64 = +sinT
    invgq2 = di("invgq2", [NQ * D, 1], BF16)  # 1/gq^2 (ssq weights)
    invgk2 = di("invgk2", [NK * D, 1], BF16)
    out = nc.dram_tensor("out", [R, H], F32, kind="ExternalOutput").ap()

    with tile.TileContext(nc) as tc:
        _emit(nc, tc, xT, wq, wk, wv, wo, cosT, sinTs, invgq2, invgk2, out)
    nc.finalize()
    return nc


def _emit(nc, tc, xT, wq, wk, wv, wo, cosT, sinTs, invgq2, invgk2, out):
    ctx = ExitStack()
    singles = ctx.enter_context(tc.tile_pool(name="singles", bufs=1))
    dram = ctx.enter_context(tc.tile_pool(name="dram", bufs=4, space="DRAM"))
    small = ctx.enter_context(tc.tile_pool(name="small", bufs=2))

    # ---- resident small inputs ----
    cos_sb = singles.tile([RD, R], BF16)
    nc.sync.dma_start(out=cos_sb, in_=cosT[:, :])
    sin_sb = singles.tile([RD, R], BF16)
    nc.sync.dma_start(out=sin_sb, in_=sinTs[:, :])
    igq_sb = singles.tile([128, NQ], BF16)
    nc.sync.dma_start(out=igq_sb, in_=invgq2.rearrange("(j p) o -> p (j o)", p=128))
    igk_sb = singles.tile([128, NK], BF16)
    nc.sync.dma_start(out=igk_sb, in_=invgk2.rearrange("(j p) o -> p (j o)", p=128))
    ident = singles.tile([128, 128], BF16)
    make_identity(nc, ident)
    eps1 = singles.tile([1, 1], F32)
    nc.vector.memset(eps1, EPS)
    ones_sb = singles.tile([128, 1], BF16)
    nc.vector.memset(ones_sb, 1.0)

    attkv = ctx.enter_context(tc.tile_pool(name="attkv", bufs=1))
    kT_full = attkv.tile([128, NK, GRP, R], BF16)     # [d, h, j, r]
    scp = ctx.enter_context(tc.tile_pool(name="scp", bufs=1))
    rk_bc = scp.tile([128, R], F32)
    rq_bc = scp.tile([128, R], F32)
    cos_q = scp.tile([RD, R], BF16)   # cos * q-norm-scale
    sin_q = scp.tile([RD, R], BF16)   # (+-)sin * q-norm-scale
    rq_b16 = scp.tile([128, R], BF16)  # q-norm-scale (rows RD:128 used)
    qpool_cm = tc.tile_pool(name="qpool", bufs=1)
    qpool = qpool_cm.__enter__()
    qts = [qpool.tile([128, R], BF16, name=f"qh{c}", tag=f"qh{c}")
           for c in range(NQ)]

    # ---- AllGather bounce buffers ----
    agk_in = dram.tile([128, NK, R], BF16, tag="agki")
    agk_out = dram.tile([GRP, 128, NK, R], BF16, tag="agko")
    agv_in = dram.tile([128, R // 128, NK, D], BF16, tag="agvi")
    agv_out = dram.tile([GRP, 128, R // 128, NK, D], BF16, tag="agvo")
    rgroups = [[0, 1, 2, 3], [4, 5, 6, 7]]

    def make_scale_bc(ssq_ps, inv_n, bc):
        """bc[128,R] = rsqrt(ssq/n + eps) via exp(-0.5*ln(x)) + gpsimd bcast."""
        r1 = small.tile([1, R], F32, tag="r1")
        nc.scalar.activation(out=r1, in_=ssq_ps,
                             func=mybir.ActivationFunctionType.Ln,
                             bias=eps1, scale=inv_n)
        r2 = small.tile([1, R], F32, tag="r2")
        nc.scalar.activation(out=r2, in_=r1,
                             func=mybir.ActivationFunctionType.Exp,
                             scale=-0.5)
        nc.gpsimd.partition_broadcast(bc, r2, channels=128)

    # =============== projections + norm + rope + AllGather ================
    with tc.tile_pool(name="kvtmp", bufs=1) as kvtmp, \
         tc.tile_pool(name="wpool", bufs=2) as wpool, \
         tc.tile_pool(name="sqpool", bufs=2) as sqpool, \
         tc.tile_pool(name="shp", bufs=1) as shp, \
         tc.tile_pool(name="pps", bufs=3, space="PSUM") as pps, \
         tc.tile_pool(name="projacc", bufs=1, space="PSUM") as projacc, \
         tc.tile_pool(name="tps", bufs=2, space="PSUM") as tps:

        kT_all = kvtmp.tile([128, NK, R], BF16)   # roped/normed in place
        V_own = kvtmp.tile([128, R // 128, NK, D], BF16)   # [p, rc, h, d]
        xT_sb = kvtmp.tile([128, NHC, R], BF16)
        xTr = xT.rearrange("(j p) r -> p j r", p=128)
        for j in range(NHC):
            nc.sync.dma_start(out=xT_sb[:, j, :], in_=xTr[:, j, :])

        pending = []

        def flush_ssq():
            while pending:
                ssq_tile, lhs, sq, first, last = pending.pop(0)
                nc.tensor.matmul(ssq_tile, lhs, sq,
                                 start=first, stop=last, skip_group_check=True)

        def proj_chunk(w_ap, c, ssq_lhs, ssq_tile, first, last, dst_ap):
            wsb = wpool.tile([128, NHC, 128], BF16, tag="wslab")
            nc.sync.dma_start(
                out=wsb,
                in_=w_ap[:, c * 128:(c + 1) * 128].rearrange("(j p) f -> p j f", p=128))
            ps = pps.tile([128, R], F32, tag="proj")
            for j in range(NHC):
                nc.tensor.matmul(ps, wsb[:, j, :], xT_sb[:, j, :],
                                 start=(j == 0), stop=(j == NHC - 1))
            nc.scalar.copy(out=dst_ap, in_=ps)
            flush_ssq()
            if ssq_tile is not None:
                sq = sqpool.tile([128, R], BF16, tag="sq")
                nc.vector.tensor_mul(sq, dst_ap, dst_ap)
                pending.append((ssq_tile, ssq_lhs[:, c:c + 1], sq, first, last))

        def rope_norm_inplace(buf_ap, cos_t, sin_t, pass_t):
            """buf = rope(buf) * s, with s pre-folded into cos_t/sin_t/pass_t."""
            sh = shp.tile([RD, R], BF16, tag="shift")
            nc.gpsimd.dma_start(out=sh[0:HALF, :], in_=buf_ap[HALF:RD, :])
            nc.gpsimd.dma_start(out=sh[HALF:RD, :], in_=buf_ap[0:HALF, :])
            nc.vector.tensor_mul(sh, sh, sin_t)
            t2 = shp.tile([RD, R], BF16, tag="tcos")
            nc.vector.tensor_mul(t2, buf_ap[0:RD, :], cos_t)
            nc.vector.tensor_add(buf_ap[0:RD, :], t2, sh)
            nc.vector.tensor_mul(buf_ap[RD:128, :], buf_ap[RD:128, :], pass_t[RD:128, :])

        # ---- k projection, norm, rope, AllGather (critical path) ----
        ssqk = projacc.tile([1, R], F32, tag="ssqk")
        for c in range(NK):
            proj_chunk(wk, c, igk_sb, ssqk, c == 0, c == NK - 1, kT_all[:, c, :])
        flush_ssq()
        make_scale_bc(ssqk, 1.0 / (NK * D), rk_bc)
        cos_k = kvtmp.tile([RD, R], BF16)
        sin_k = kvtmp.tile([RD, R], BF16)
        rk_b16 = kvtmp.tile([128, R], BF16)
        nc.vector.tensor_mul(cos_k, cos_sb, rk_bc[0:RD, :])
        nc.vector.tensor_mul(sin_k, sin_sb, rk_bc[0:RD, :])
        nc.vector.tensor_copy(rk_b16, rk_bc)
        for c in range(NK):
            rope_norm_inplace(kT_all[:, c, :], cos_k, sin_k, rk_b16)
        nc.gpsimd.dma_start(out=agk_in[:], in_=kT_all)
        nc.gpsimd.collective_compute(
            "AllGather", mybir.AluOpType.bypass, replica_groups=rgroups,
            ins=[agk_in.opt()], outs=[agk_out.opt()])

        # ---- v projection (overlaps k rope / collective) ----
        for c in range(NK):
            vt = sqpool.tile([128, R], BF16, tag="vtchunk")
            proj_chunk(wv, c, None, None, False, False, vt[:, :])
            for rc in range(R // 128):
                tp = tps.tile([128, 128], BF16, tag="vtp")
                nc.tensor.transpose(tp, vt[:, rc * 128:(rc + 1) * 128], ident)
                nc.scalar.copy(out=V_own[:, rc, c, :], in_=tp)
        nc.gpsimd.dma_start(out=agv_in[:], in_=V_own)
        nc.gpsimd.collective_compute(
            "AllGather", mybir.AluOpType.bypass, replica_groups=rgroups,
            ins=[agv_in.opt()], outs=[agv_out.opt()])
        for j in range(GRP):
            nc.sync.dma_start(out=kT_full[:, :, j, :], in_=agk_out[j])

        # ---- q projection ----
        ssqq = projacc.tile([1, R], F32, tag="ssqq")
        for c in range(NQ):
            proj_chunk(wq, c, igq_sb, ssqq, c == 0, c == NQ - 1, qts[c][:, :])
        flush_ssq()
        make_scale_bc(ssqq, 1.0 / (NQ * D), rq_bc)
        # fold the q norm scale into the rope tables (all bf16 pairs after)
        nc.vector.tensor_mul(cos_q, cos_sb, rq_bc[0:RD, :])
        nc.vector.tensor_mul(sin_q, sin_sb, rq_bc[0:RD, :])
        nc.vector.tensor_copy(rq_b16, rq_bc)
        # q rope happens per-head inside the attention loop.

    # ========================== attention =================================
    aots = [singles.tile([128, R], BF16, name=f"ao{c}", tag=f"ao{c}")
            for c in range(NQ)]
    GB = 2                    # score chunks per exp batch
    NG = KC // GB             # 8 exp groups per head
    with tc.tile_pool(name="vfp", bufs=1) as vfp, \
         tc.tile_pool(name="expp", bufs=2) as expp, \
         tc.tile_pool(name="tr1p", bufs=1) as tr1p, \
         tc.tile_pool(name="tr2p", bufs=1) as tr2p, \
         tc.tile_pool(name="shq", bufs=2) as shq, \
         tc.tile_pool(name="attps", bufs=2, space="PSUM") as attps, \
         tc.tile_pool(name="accps", bufs=2, space="PSUM") as accps, \
         tc.tile_pool(name="sump", bufs=2, space="PSUM") as sump:
        V_full = vfp.tile([128, KC, NK, D], BF16)     # [p, kc, h, d]
        for j in range(GRP):
            nc.sync.dma_start(
                out=V_full[:, j * (R // 128):(j + 1) * (R // 128), :, :],
                in_=agv_out[j])

        def rope_q(qh):
            sh = shq.tile([RD, R], BF16, tag="qshift")
            nc.gpsimd.dma_start(out=sh[0:HALF, :], in_=qts[qh][HALF:RD, :])
            nc.gpsimd.dma_start(out=sh[HALF:RD, :], in_=qts[qh][0:HALF, :])
            nc.vector.tensor_mul(sh, sh, sin_q)
            t2 = shq.tile([RD, R], BF16, tag="qcos")
            nc.vector.tensor_mul(t2, qts[qh][0:RD, :], cos_q)
            nc.vector.tensor_add(qts[qh][0:RD, :], t2, sh)
            nc.vector.tensor_mul(qts[qh][RD:128, :], qts[qh][RD:128, :], rq_b16[RD:128, :])

        # per-head state carried into the next head's block (sw pipeline)
        carry = {}

        def head_tail_a(hh):
            """vector tree levels 2-4 for head hh (L1 at end of hh's block)."""
            st = carry[hh]
            tr1 = st["tr1"]
            tr2 = tr2p.tile([128, 2048 + 1024 + 512], BF16, tag="tr2")
            nc.vector.tensor_add(tr2[:, 0:2048], tr1[:, 0:2048], tr1[:, 2048:4096])
            nc.vector.tensor_add(tr2[:, 2048:3072], tr2[:, 0:1024], tr2[:, 1024:2048])
            nc.vector.tensor_add(tr2[:, 3072:3584], tr2[:, 2048:2560], tr2[:, 2560:3072])
            st["tr2"] = tr2

        def head_tail_b(hh):
            """sum matmul for head hh (rank-1)."""
            st = carry[hh]
            sum_ps = sump.tile([1, R], F32, tag="sumexp")
            nc.tensor.matmul(sum_ps, ones_sb, st["tr2"][:, 3072:3584],
                             start=True, stop=True, skip_group_check=True)
            st["sum_ps"] = sum_ps

        def head_tail_c(hh):
            """reciprocal + broadcast + aot normalize for head hh."""
            st = carry[hh]
            rec = small.tile([1, R], F32, tag="rec")
            if hh % 2 == 0:
                nc.vector.reciprocal(out=rec, in_=st["sum_ps"])
            else:
                # scalar-engine 1/x = exp(-ln(x)) to offload the DVE
                lnt = small.tile([1, R], F32, tag="lnt")
                nc.scalar.activation(out=lnt, in_=st["sum_ps"],
                                     func=mybir.ActivationFunctionType.Ln)
                nc.scalar.activation(out=rec, in_=lnt,
                                     func=mybir.ActivationFunctionType.Exp,
                                     scale=-1.0)
            rec_bc = tr2p.tile([128, R], F32, tag="rec_bc")
            nc.gpsimd.partition_broadcast(rec_bc, rec, channels=128)
            nc.vector.tensor_mul(aots[hh][:, :], st["out_ps"], rec_bc)
            del carry[hh]

        rope_q(0)
        for qh in range(NQ):
            kvh = qh // GRP
            if qh + 1 < NQ:
                rope_q(qh + 1)
            if qh >= 1:
                head_tail_a(qh - 1)
            ex = expp.tile([128, KC, R], BF16, tag="expT")
            out_ps = accps.tile([128, R], F32, tag="pv")
            st = {"ex": ex, "out_ps": out_ps}
            carry[qh] = st
            for g in range(NG):
                sps = attps.tile([128, GB * R], F32, tag="scoresT")
                for i in range(GB):
                    kc = g * GB + i
                    nc.tensor.matmul(
                        sps[:, i * R:(i + 1) * R],
                        kT_full[:, kvh, kc // 4, (kc % 4) * 128:(kc % 4) * 128 + 128],
                        qts[qh][:, :], start=True, stop=True,
                        skip_group_check=True)
                nc.scalar.activation(
                    out=ex[:, g * GB:(g + 1) * GB, :], in_=sps,
                    func=mybir.ActivationFunctionType.Exp, scale=SCALE)
                if g == 2 and qh >= 1:
                    head_tail_b(qh - 1)
                if g == 3 and qh >= 1:
                    head_tail_c(qh - 1)
                if g >= 1:
                    for i in range(GB):
                        kc = (g - 1) * GB + i
                        nc.tensor.matmul(out_ps, V_full[:, kc, kvh, :],
                                         ex[:, kc, :], start=(kc == 0),
                                         stop=False, skip_group_check=True)
            for i in range(GB):
                kc = (NG - 1) * GB + i
                nc.tensor.matmul(out_ps, V_full[:, kc, kvh, :], ex[:, kc, :],
                                 start=False, stop=(kc == KC - 1),
                                 skip_group_check=True)
            # tree level 1: [128, 8192] -> [128, 4096]
            tr1 = tr1p.tile([128, 4096], BF16, tag="tr1")
            nc.vector.tensor_add(tr1, ex[:, 0:KC // 2, :], ex[:, KC // 2:KC, :])
            st["tr1"] = tr1
        head_tail_a(NQ - 1)
        head_tail_b(NQ - 1)
        head_tail_c(NQ - 1)

    qpool_cm.__exit__(None, None, None)

    # ======================= output projection ============================
    # attn_oT stationary across all H-blocks of a pair: 128 LDWs total.
    with tc.tile_pool(name="wop", bufs=2) as wop, \
         tc.tile_pool(name="odr", bufs=4) as odr, \
         tc.tile_pool(name="ops", bufs=2, space="PSUM") as ops:
        NHP = H // 512
        for hp in range(NHP):
            wosb = wop.tile([128, NQ, 512], BF16, tag="wo")
            wor = wo[:, hp * 512:(hp + 1) * 512].rearrange(
                "(j p) f -> p j f", p=128)
            for hq in range(0, NQ, 8):
                nc.sync.dma_start(out=wosb[:, hq:hq + 8, :],
                                  in_=wor[:, hq:hq + 8, :])
            pos = [ops.tile([128, 512], F32, tag=f"ops{i}", name=f"po{hp}_{i}")
                   for i in range(4)]
            for h in range(NQ):
                for rc in range(R // 128):
                    nc.tensor.matmul(
                        pos[rc],
                        aots[h][:, rc * 128:(rc + 1) * 128],
                        wosb[:, h, :],
                        start=(h == 0), stop=(h == NQ - 1),
                        skip_group_check=True)
            for rc in range(R // 128):
                osb = odr.tile([128, 512], F32, tag="odr")
                nc.scalar.copy(out=osb, in_=pos[rc])
                nc.sync.dma_start(
                    out=out[rc * 128:(rc + 1) * 128, hp * 512:(hp + 1) * 512],
                    in_=osb)
    ctx.close()


def _get_nc():
    if "nc" not in _cache:
        _cache["nc"] = _build()
    return _cache["nc"]


def kernel(x, cos, sin, wq, wk, wv, wo, gq, gk):
    bf = ml_dtypes.bfloat16
    x = np.asarray(x, np.float32)
    cos = np.asarray(cos, np.float32)
    sin = np.asarray(sin, np.float32)
    gq = np.asarray(gq, np.float32)
    gk = np.asarray(gk, np.float32)
    wqp = (np.asarray(wq, np.float32) * gq[None, :]).astype(bf)
    wkp = (np.asarray(wk, np.float32) * gk[None, :]).astype(bf)
    wv_b = np.asarray(wv, np.float32).astype(bf)
    wo_b = np.asarray(wo, np.float32).astype(bf)
    igq = np.where(gq == 0, 0, 1.0 / np.maximum(gq * gq, 1e-30)).astype(bf)[:, None]
    igk = np.where(gk == 0, 0, 1.0 / np.maximum(gk * gk, 1e-30)).astype(bf)[:, None]

    x2 = x.reshape(B * S, H)
    in_maps = []
    for c in range(NCORES):
        p0 = (c % GRP) * R
        sinT = sin[p0:p0 + R].T.astype(np.float32)        # [RD, R]
        sinTs = np.concatenate([-sinT[:HALF], sinT[HALF:]], 0)
        in_maps.append({
            "xT": np.ascontiguousarray(x2[c * R:(c + 1) * R].T).astype(bf),
            "wq": wqp, "wk": wkp, "wv": wv_b, "wo": wo_b,
            "cosT": np.ascontiguousarray(cos[p0:p0 + R].T).astype(bf),
            "sinTs": np.ascontiguousarray(sinTs).astype(bf),
            "invgq2": igq, "invgk2": igk,
        })
    nc = _get_nc()
    import os
    kw = {}
    if os.environ.get("KERNEL_TRACE"):
        kw = dict(trace=True, tmpdir=os.environ.get("KERNEL_TRACE_DIR") or None)
    res = run_bass_kernel_spmd(nc, in_maps, core_ids=list(range(NCORES)), **kw)
    kernel.last_exec_time_ns = res.exec_time_ns
    outp = np.concatenate([res.results[c]["out"] for c in range(NCORES)], 0)
    return outp.reshape(B, S, H).astype(np.float32)
